# revision 1
# baseline (speedup 1.0000x reference)
"""DeBut 2D-conv kernel for Trainium2 (8 NeuronCores, data-parallel over batch).

Math: the reference is im2col(x) -> chain of 3 deformable-butterfly factors
-> +bias -> reshape.  The three factors compose into a single block-diagonal
matrix M (256x1152): M[o, i] != 0 only for i in [18*(o//4), 18*(o//4)+18).
With im2col feature order (kh, kw, c), feature chunk kk*128..kk*128+128 of a
pixel (h, w) is just x[:, h+kh-1, w+kw-1] -- a spatially shifted channel
vector.  So conv == 9 shifted [128 x 128] matmuls accumulated in PSUM, and
because of the band structure each 128-feature chunk only touches one or two
128-output-channel halves: 10 (chunk, half) pairs have nonzero weights.

Per core: 2 images; x is zero-padded to 58x58 on host (so shifts are exact
strided views of one SBUF tile) and cast to bf16; weights composed on host in
float64 and cast to bf16; accumulation is fp32 in PSUM.
"""

import numpy as np
import ml_dtypes

import concourse.bass as bass
import concourse.tile as tile
from concourse import bacc, mybir
from concourse.bass_utils import run_bass_kernel_spmd

# Problem constants (hardcoded; kernel.py must be self-contained).
B, C_IN, H, W = 16, 128, 56, 56
C_OUT = 256
HP, WP = H + 2, W + 2  # zero-padded spatial dims (58, 58)
N_CORES = 8
B_CORE = B // N_CORES  # 2 images per core
R_SHAPES = [[512, 1152, 4, 9, 1], [512, 512, 4, 4, 1], [256, 512, 2, 4, 2]]

ROWS_PER_TILE = 8            # 8 rows x 56 cols = 448 pixels per PSUM tile
NT = H // ROWS_PER_TILE      # 7 pixel tiles per image
FREE = ROWS_PER_TILE * W     # 448 <= 512 fp32 per PSUM bank

# (m, kk) pairs with a nonzero weight band: m = output-channel half (0/1),
# kk = kh*3+kw 3x3 tap index.  Feature chunk kk covers im2col features
# [128kk, 128kk+128) -> blocks k3 in ~[7.1kk, 7.1kk+7.1] -> channels 4*k3.
PAIRS = [(0, 0), (0, 1), (0, 2), (0, 3), (0, 4),
         (1, 4), (1, 5), (1, 6), (1, 7), (1, 8)]
KKS = {0: [0, 1, 2, 3, 4], 1: [4, 5, 6, 7, 8]}
JIDX = {pair: j for j, pair in enumerate(PAIRS)}

# every matmul writes the full 128-partition output chunk (band weights
# zero-padded to M=128): simplest exact PSUM accumulate/overwrite semantics
WIN = {pair: (0, 128) for pair in PAIRS}
WCOL = {pair: JIDX[pair] * 128 for pair in PAIRS}
W_COLS = len(PAIRS) * 128  # 1280

BF16 = mybir.dt.bfloat16
F32 = mybir.dt.float32

_CACHE = {}


def _debut_matrix(twiddle: np.ndarray) -> np.ndarray:
    """Compose the butterfly chain into M (256x1152) with out = M @ x."""
    out = np.eye(1152, dtype=np.float64)
    p = 0
    for (out_size, in_size, row, col, diag) in R_SHAPES:
        num_p = col * out_size
        blocks = in_size // (col * diag)
        t = (twiddle[p:p + num_p].astype(np.float64)
             .reshape(blocks, diag, row, col).transpose(0, 2, 3, 1))
        xr = out.reshape(-1, blocks, col, diag)
        out = np.einsum('krcd,nkcd->nkrd', t, xr).reshape(-1, out_size)
        p += num_p
    return out.T  # (256, 1152)


def _build_nc(repeat: int = 1, probe: str = "") -> bacc.Bacc:
    """repeat > 1 wraps the whole compute body in a device-side For_i loop
    (used only by the timing harness; the graded path uses repeat=1).
    probe='peonly' strips DMA/evacuation to measure the pure matmul stream."""
    nc = bacc.Bacc("TRN2", target_bir_lowering=False, debug=False,
                   num_devices=N_CORES)
    xd = nc.dram_tensor("xpad", [B_CORE, C_IN, HP, WP], BF16,
                        kind="ExternalInput")
    wd = nc.dram_tensor("wmat", [C_IN, W_COLS], BF16,
                        kind="ExternalInput")
    bd = nc.dram_tensor("bias2", [128, 2], F32, kind="ExternalInput")
    yd = nc.dram_tensor("y", [B_CORE, C_OUT, H, W], BF16,
                        kind="ExternalOutput")

    with tile.TileContext(nc) as tc:
        with (
            tc.tile_pool(name="wpool", bufs=1) as wpool,
            tc.tile_pool(name="bpool", bufs=1) as bpool,
            tc.tile_pool(name="xpool", bufs=3) as xpool,
            tc.tile_pool(name="opool", bufs=6) as opool,
            tc.tile_pool(name="psum", bufs=8, space="PSUM") as ppool,
        ):
            # the first matmul needs only weight pair j=0: load those cols
            # first (scalar HWDGE queue, parallel with x on sync)
            w_split = WIN[PAIRS[0]][1]
            w_t = wpool.tile([C_IN, W_COLS], BF16)
            nc.scalar.dma_start(w_t[:, :w_split], wd.ap()[:, :w_split])
            nc.scalar.dma_start(w_t[:, w_split:], wd.ap()[:, w_split:])
            bias_t = bpool.tile([128, 2], F32)

            # x-load row chunks (padded rows incl. halo): first chunk covers
            # just pixel-tile 0 so the matmul stream starts ~0.5us in
            X_CHUNKS = [(0, 12), (12, 34), (34, HP)]
            # store-chunk end-tile -> start-tile: chunks of 4, 2, then 1 tile
            STORE_BOUNDARIES = {3: 0, 5: 4, 6: 6}

            def load_x(b):
                xp_t = xpool.tile([C_IN, HP, WP], BF16, name=f"xp_{b}",
                                  tag="xp")
                for r0, r1 in X_CHUNKS:
                    nc.sync.dma_start(xp_t[:, r0:r1, :],
                                      xd.ap()[b, :, r0:r1, :])
                return xp_t

            def body():
                for b in range(B_CORE):
                    xp_t = load_x(b)
                    if b == 0:
                        nc.scalar.dma_start(bias_t[:], bd.ap()[:])
                    o_img = {}
                    if probe != "peonly":
                        for m in range(2):
                            o_img[m] = opool.tile([128, NT, FREE], BF16,
                                                  name=f"o_img_{b}_{m}",
                                                  tag="o_img")
                    for t in range(NT):
                        for m in range(2):
                            ps = ppool.tile([128, FREE], F32)
                            kks = KKS[m]
                            for i, kk in enumerate(kks):
                                kh, kw = divmod(kk, 3)
                                base, msize = WIN[(m, kk)]
                                col = WCOL[(m, kk)]
                                rhs = xp_t[:, t * 8 + kh: t * 8 + kh + 8,
                                           kw: kw + W]
                                nc.tensor.matmul(
                                    ps[base:base + msize, :],
                                    w_t[:, col:col + msize], rhs,
                                    start=(i == 0), stop=(i == len(kks) - 1))
                            if probe == "peonly":
                                continue
                            # split PSUM evacuation across ACT and DVE; whole
                            # image-half accumulates in SBUF so stores are a
                            # few big descriptors per partition
                            if m == 0:
                                nc.scalar.add(o_img[m][:, t, :], ps[:],
                                              bias_t[:, m:m + 1])
                            else:
                                nc.vector.tensor_scalar_add(
                                    o_img[m][:, t, :], ps[:],
                                    bias_t[:, m:m + 1])
                        # issue each store chunk as soon as its tiles are
                        # done; the last chunk is a single tile to keep the
                        # kernel tail short
                        if t in STORE_BOUNDARIES and probe != "peonly":
                            t0 = STORE_BOUNDARIES[t]
                            for m in range(2):
                                eng = nc.sync if m == 0 else nc.scalar
                                eng.dma_start(
                                    yd.ap()[b, m * 128:(m + 1) * 128,
                                            t0 * 8:(t + 1) * 8, :],
                                    o_img[m][:, t0:t + 1, :])

            # Warmup matmuls on a scratch tile during the DMA-load head: the
            # PE HAM activity window starts seeing a busy PE at t~0, so the
            # 1.2->2.4 GHz un-throttle fires ~1-2us earlier than if the first
            # real matmul (gated on the x DMA) started the clock.
            wm_src = wpool.tile([C_IN, 64], BF16, name="wm_src")
            nc.vector.memset(wm_src[:], 0.0)
            wm_ps = ppool.tile([64, 64], F32, name="wm_ps", tag="ps")
            for _ in range(16):
                nc.tensor.matmul(wm_ps[:], wm_src[:, :64], wm_src[:, :64],
                                 start=True, stop=True)

            if repeat == 1:
                body()
            else:
                with tc.For_i(0, repeat, 1,
                              hint_engines=(mybir.EngineType.PE,
                                            mybir.EngineType.Activation,
                                            mybir.EngineType.SP)):
                    body()
    nc.finalize()
    return nc


def _prep_inputs(x: np.ndarray, twiddle: np.ndarray, bias: np.ndarray):
    """Host-side: pad + cast x, compose weights, arrange per-core in_maps."""
    x = np.asarray(x, dtype=np.float32)
    xpad = np.zeros((B, C_IN, HP, WP), dtype=ml_dtypes.bfloat16)
    xpad[:, :, 1:1 + H, 1:1 + W] = x.astype(ml_dtypes.bfloat16)

    M = _debut_matrix(np.asarray(twiddle, dtype=np.float32))
    wmat = np.zeros((C_IN, W_COLS), dtype=np.float64)
    for (m, kk) in PAIRS:
        # lhsT layout: wmat[c, WCOL + i] = M[128m + base + i, 128kk + c]
        base, msize = WIN[(m, kk)]
        col = WCOL[(m, kk)]
        wmat[:, col:col + msize] = M[m * 128 + base:m * 128 + base + msize,
                                     kk * 128:(kk + 1) * 128].T
    wmat = wmat.astype(ml_dtypes.bfloat16)

    bias2 = np.asarray(bias, dtype=np.float32).reshape(2, 128).T.copy()

    in_maps = []
    for core in range(N_CORES):
        in_maps.append({
            "xpad": xpad[core * B_CORE:(core + 1) * B_CORE],
            "wmat": wmat,
            "bias2": bias2,
        })
    return in_maps


def kernel(x: np.ndarray, twiddle: np.ndarray, bias: np.ndarray) -> np.ndarray:
    if "nc" not in _CACHE:
        _CACHE["nc"] = _build_nc()
    nc = _CACHE["nc"]
    in_maps = _prep_inputs(x, twiddle, bias)
    res = run_bass_kernel_spmd(nc, in_maps, list(range(N_CORES)))
    out = np.concatenate(
        [np.asarray(res.results[i]["y"]) for i in range(N_CORES)], axis=0)
    return np.ascontiguousarray(out.astype(np.float32))

